# revision 1
# baseline (speedup 1.0000x reference)
"""MultiHeadAttention (B=4, N=2048, E=1024, H=16) on 8 TRN2 NeuronCores.

Sharding: core c handles batch b = c//2 and head-half hh = c%2 (8 heads,
512 embed dims). Each core computes Q/K/V projections for its 8 heads,
attention, and a partial output projection (contraction over its 512 c-dims).
Host sums the two partials per batch and adds the output bias.

All matmuls run as float32r (tf32 mantissa, fp32 accumulate) at full PE rate.
Layouts are chosen so no transposes are ever needed on device:
  - host ships x.T  [embed, tok] so projections contract embed on partitions
  - Q/K are produced transposed: QT/KT [dout, tok]
  - scores are computed directly as S.T [k, q] (contraction d<=64)
  - V is produced in natural [tok, dv] layout with a ones-column appended per
    head, so attn@V yields O.T [d, q] AND the softmax denominators in row 64
  - softmax skips max-subtraction (|scores/8| < ~3, exp is safe in fp32)
  - output projection consumes O.T directly; host transposes the result once
"""
import sys

sys.path.insert(0, "/opt/trn_rl_repo")

import numpy as np

B, N, E = 4, 2048, 1024
NCORES = 8
HH = 512          # embed dims (8 heads x 64) per core
D = 64
NHEAD = 8         # heads per core

_cache = {}


def _tf32(x):
    u = np.ascontiguousarray(x, dtype=np.float32).view(np.uint32)
    lsb = (u >> 13) & 1
    u = (u + 0x0FFF + lsb) & 0xFFFFE000
    return u.view(np.float32)


def _split_matmul_waits(nc, mybir):
    """fp32r self-loading matmuls cannot carry sync waits (walrus places
    them on the S3_LW struct which has no wait slot). Move every wait off
    Matmult instructions onto InstEventSemaphore instructions inserted
    just before, in block order."""
    n_fixed = 0
    for fn in nc.m.functions:
        for blk in fn.blocks:
            insts = blk.instructions
            i = 0
            while i < len(insts):
                inst = insts[i]
                si = inst.sync_info
                if inst.opcode == "Matmult" and si is not None and len(si.on_wait) > 0:
                    waits = list(si.on_wait)
                    si.on_wait = []
                    inst.sync_info = si
                    pos = i
                    for j in range(0, len(waits), 2):
                        ev = mybir.InstEventSemaphore(
                            name=f"mmgate_{inst.name}_{j}",
                            ins=[],
                            outs=[],
                            sync_info=mybir.SyncInfo(
                                on_wait=waits[j : j + 2], on_update=[]
                            ),
                        )
                        ev.engine = inst.engine
                        nc.register_instruction(ev)
                        insts.insert(pos, ev)
                        pos += 1
                        i += 1
                    n_fixed += 1
                i += 1
            blk.instructions = insts
    return n_fixed


def _build():
    import concourse.mybir as mybir
    import concourse.tile as tile
    import concourse.bacc as bacc

    F32 = mybir.dt.float32
    F32R = mybir.dt.float32r
    EXP = mybir.ActivationFunctionType.Exp

    nc = bacc.Bacc(trn_type="TRN2")

    xtq = nc.dram_tensor("xtq", [E, N], F32R, kind="ExternalInput")
    xtk = nc.dram_tensor("xtk", [E, N], F32R, kind="ExternalInput")
    xtv = nc.dram_tensor("xtv", [E, N], F32R, kind="ExternalInput")
    wqt = nc.dram_tensor("wqt", [E, HH], F32R, kind="ExternalInput")
    wkt = nc.dram_tensor("wkt", [E, HH], F32R, kind="ExternalInput")
    wvt = nc.dram_tensor("wvt", [E, HH], F32R, kind="ExternalInput")
    wot = nc.dram_tensor("wot", [HH, E], F32R, kind="ExternalInput")
    bq = nc.dram_tensor("bq", [HH], F32, kind="ExternalInput")
    bk = nc.dram_tensor("bk", [HH], F32, kind="ExternalInput")
    bv = nc.dram_tensor("bv", [HH], F32, kind="ExternalInput")
    po = nc.dram_tensor("po", [E, N], F32, kind="ExternalOutput")

    with tile.TileContext(nc) as tc:
        with (
            tc.tile_pool(name="consts", bufs=1) as consts,
            tc.tile_pool(name="qk", bufs=1) as qk_pool,
            tc.tile_pool(name="vx", bufs=1) as v_pool,
            tc.tile_pool(name="wo", bufs=1) as wo_pool,
        ):
            # ---------------- constants ----------------
            ones_f = consts.tile([1, 128], F32)
            nc.vector.memset(ones_f, 1.0)
            ones_r = consts.tile([1, 128], F32R)
            nc.vector.tensor_copy(ones_r, ones_f)
            onescol_f = consts.tile([128, NHEAD, 1], F32)
            nc.vector.memset(onescol_f, 1.0)

            bq_t = consts.tile([128, 4], F32)
            bk_t = consts.tile([128, 4], F32)
            nc.sync.dma_start(out=bq_t, in_=bq.ap().rearrange("(t p) -> p t", p=128))
            nc.sync.dma_start(out=bk_t, in_=bk.ap().rearrange("(t p) -> p t", p=128))
            bv_row = consts.tile([1, HH], F32)
            nc.sync.dma_start(out=bv_row, in_=bv.ap().rearrange("(a n) -> a n", a=1))
            bv_row_r = consts.tile([1, HH], F32R)
            nc.vector.tensor_copy(bv_row_r, bv_row)
            bv_bc = consts.tile([128, HH], F32)

            # persistent activations
            QT = [qk_pool.tile([128, N], F32R, tag=f"qt{t}", name=f"qt{t}") for t in range(4)]
            KT = [qk_pool.tile([128, N], F32R, tag=f"kt{t}", name=f"kt{t}") for t in range(4)]
            VE = [v_pool.tile([128, NHEAD, D + 1], F32R, tag=f"ve{g}", name=f"ve{g}") for g in range(16)]
            wo_t = wo_pool.tile([128, 4, E], F32R, tag="wo")

            # ---------------- projections ----------------
            with (
                tc.tile_pool(name="w", bufs=2) as w_pool,
                tc.tile_pool(name="xt", bufs=2) as xt_pool,
                tc.tile_pool(name="pps", bufs=4, space="PSUM") as proj_ps,
            ):
                # broadcast bv to all partitions via K=1 matmul
                bc0 = proj_ps.tile([128, HH], F32, tag="bvbc")
                nc.tensor.matmul(bc0, ones_r, bv_row_r, start=True, stop=True)
                nc.vector.tensor_copy(bv_bc, bc0)

                w_tiles = {}
                for name, wdram in (("q", wqt), ("k", wkt), ("v", wvt)):
                    wt = w_pool.tile([128, 8, HH], F32R, tag="w")
                    nc.sync.dma_start(
                        out=wt, in_=wdram.ap().rearrange("(kt p) n -> p kt n", p=128)
                    )
                    w_tiles[name] = wt

                def qk_proj(xdram, wt, dest, bias_t):
                    for th in range(4):
                        xt = xt_pool.tile([128, 8, 512], F32R, tag="xt")
                        nc.sync.dma_start(
                            out=xt,
                            in_=xdram.ap().rearrange("(kt p) n -> p kt n", p=128)[
                                :, :, 512 * th : 512 * (th + 1)
                            ],
                        )
                        for dt_ in range(4):
                            ps = proj_ps.tile([128, 512], F32, tag="pp")
                            for kt in range(8):
                                nc.tensor.matmul(
                                    ps,
                                    wt[:, kt, 128 * dt_ : 128 * (dt_ + 1)],
                                    xt[:, kt, :],
                                    start=(kt == 0),
                                    stop=(kt == 7),
                                )
                            off = 512 * th
                            nc.vector.tensor_scalar_add(
                                dest[dt_][:, off : off + 512],
                                ps,
                                bias_t[:, dt_ : dt_ + 1],
                            )

                qk_proj(xtq, w_tiles["q"], QT, bq_t)
                qk_proj(xtk, w_tiles["k"], KT, bk_t)

                # V in natural [tok, dv] layout + ones column
                for th in range(4):
                    xt = xt_pool.tile([128, 8, 512], F32R, tag="xt")
                    nc.sync.dma_start(
                        out=xt,
                        in_=xtv.ap().rearrange("(kt p) n -> p kt n", p=128)[
                            :, :, 512 * th : 512 * (th + 1)
                        ],
                    )
                    for tt in range(4):
                        g = 4 * th + tt
                        ps = proj_ps.tile([128, 512], F32, tag="pp")
                        for kt in range(8):
                            nc.tensor.matmul(
                                ps,
                                xt[:, kt, 128 * tt : 128 * (tt + 1)],
                                w_tiles["v"][:, kt, :],
                                start=(kt == 0),
                                stop=(kt == 7),
                            )
                        nc.vector.tensor_add(
                            VE[g][:, :, 0:D],
                            ps.rearrange("p (h d) -> p h d", h=NHEAD),
                            bv_bc.rearrange("p (h d) -> p h d", h=NHEAD),
                        )
                        nc.vector.tensor_copy(VE[g][:, :, D : D + 1], onescol_f)

                # output projection weights (loaded during attention DMA slack)
                nc.sync.dma_start(
                    out=wo_t, in_=wot.ap().rearrange("(ct p) n -> p ct n", p=128)
                )

            # ---------------- attention ----------------
            with (
                tc.tile_pool(name="attn", bufs=5) as attn_pool,
                tc.tile_pool(name="otn", bufs=1) as otn_pool,
                tc.tile_pool(name="small", bufs=2) as small_pool,
                tc.tile_pool(name="ostage", bufs=2) as ostage_pool,
                tc.tile_pool(name="st_ps", bufs=1, space="PSUM") as st_ps,
                tc.tile_pool(name="ot_ps", bufs=2, space="PSUM") as ot_ps,
                tc.tile_pool(name="bc_ps", bufs=1, space="PSUM") as bc_ps,
                tc.tile_pool(name="oj_ps", bufs=1, space="PSUM") as oj_ps,
            ):
                for qb in range(4):
                    q0 = 512 * qb
                    otn = [
                        otn_pool.tile([128, 512], F32R, tag=f"otn{ct}",
                                      name=f"otn{ct}_{qb}")
                        for ct in range(4)
                    ]
                    for h in range(NHEAD):
                        t, par = h // 2, (h % 2) * 64
                        at_tiles = []
                        for g in range(4):
                            stg = st_ps.tile([128, 2048], F32, tag="st")
                            for kg in range(4):
                                kt = 4 * g + kg
                                nc.tensor.matmul(
                                    stg[:, 512 * kg : 512 * (kg + 1)],
                                    KT[t][par : par + 64, 128 * kt : 128 * (kt + 1)],
                                    QT[t][par : par + 64, q0 : q0 + 512],
                                    start=True,
                                    stop=True,
                                )
                            at_g = attn_pool.tile([128, 4, 512], F32R, tag="attnT")
                            nc.scalar.activation(at_g, stg, EXP, scale=0.125)
                            at_tiles.append(at_g)
                        ot = ot_ps.tile([128, 512], F32, tag="ot")
                        for kt in range(16):
                            nc.tensor.matmul(
                                ot[0:65, :],
                                VE[kt][:, h, :],
                                at_tiles[kt // 4][:, kt % 4, :],
                                start=(kt == 0),
                                stop=(kt == 15),
                            )
                        r = small_pool.tile([1, 512], F32R, tag="recip")
                        with nc.allow_low_precision(reason="tf32 softmax denom"):
                            nc.vector.reciprocal(r, ot[64:65, :])
                        bc = bc_ps.tile([128, 512], F32, tag="bc")
                        nc.tensor.matmul(
                            bc[0:64, :], ones_r[:, 0:64], r, start=True, stop=True
                        )
                        rbc = small_pool.tile([64, 512], F32, tag="rbc")
                        nc.vector.tensor_copy(rbc, bc[0:64, :])
                        nc.vector.tensor_mul(
                            otn[t][par : par + 64, :], ot[0:64, :], rbc
                        )
                    # output projection for this q-block
                    for jt in range(8):
                        pj = oj_ps.tile([128, 512], F32, tag="oj")
                        for ct in range(4):
                            nc.tensor.matmul(
                                pj,
                                wo_t[:, ct, 128 * jt : 128 * (jt + 1)],
                                otn[ct],
                                start=(ct == 0),
                                stop=(ct == 3),
                            )
                        oj_sb = ostage_pool.tile([128, 512], F32, tag="oj_sb")
                        nc.vector.tensor_copy(oj_sb, pj)
                        nc.sync.dma_start(
                            out=po.ap()[128 * jt : 128 * (jt + 1), q0 : q0 + 512],
                            in_=oj_sb,
                        )

    nc.compile()
    _split_matmul_waits(nc, mybir)
    return nc


def _get_nc():
    if "nc" not in _cache:
        _cache["nc"] = _build()
    return _cache["nc"]


def kernel(query, key, value, Wq, bq, Wk, bk, Wv, bv, Wo, bo):
    from concourse.bass_utils import run_bass_kernel_spmd

    nc = _get_nc()

    query = np.asarray(query, dtype=np.float32)
    key = np.asarray(key, dtype=np.float32)
    value = np.asarray(value, dtype=np.float32)
    Wq, Wk, Wv, Wo = (np.asarray(w, dtype=np.float32) for w in (Wq, Wk, Wv, Wo))
    bq, bk, bv, bo = (np.asarray(b, dtype=np.float32) for b in (bq, bk, bv, bo))

    in_maps = []
    for c in range(NCORES):
        b, hh = c // 2, c % 2
        cols = slice(HH * hh, HH * (hh + 1))
        in_maps.append(
            {
                "xtq": _tf32(query[b].T),
                "xtk": _tf32(key[b].T),
                "xtv": _tf32(value[b].T),
                "wqt": _tf32(Wq[cols, :].T),
                "wkt": _tf32(Wk[cols, :].T),
                "wvt": _tf32(Wv[cols, :].T),
                "wot": _tf32(Wo[:, cols].T),
                "bq": bq[cols],
                "bk": bk[cols],
                "bv": bv[cols],
            }
        )

    _cache["in_maps"] = in_maps
    res = run_bass_kernel_spmd(nc, in_maps, core_ids=list(range(NCORES)))
    out = np.empty((B, N, E), dtype=np.float32)
    for b in range(B):
        p = res.results[2 * b]["po"] + res.results[2 * b + 1]["po"]
        out[b] = p.T + bo
    return out

